# revision 1
# baseline (speedup 1.0000x reference)
"""Causal self-attention (B=4, T=2048, E=1024, H=16, D=64) on 8 TRN2 NeuronCores.

Sharding: core c -> batch b=c//2, head-group g=c%2 (8 heads each).

v4 = v3 + restored 1-block S/exp lookahead (S of block kb+1 is emitted before
the AV of block kb, so the in-order PE queue never stalls on exp) + static
fine-grained interleave: projection (A) and output-proj (D) work is emitted
as ~0.85us units woven between attention key-blocks, keeping the PE dense
while ScalarE runs exp. Weight loads are m-major so the first projection
chain starts after ~1/4 of Wq.

q/k are stored fp8e4 in DoubleRow layout [32p, 2j, T] (dim 32j+p), giving 2x
PE rate on S^T = k^T q with the 1/8 softmax scale folded into q at quantize
time. exp on ScalarE (fp32r out), AV with vaug stationary / scores moving,
denominator via ones column + gpsimd partition_broadcast.
Host: out[b] = partial[2b] + partial[2b+1] + bp.
"""
import sys

if '/opt/trn_rl_repo' not in sys.path:
    sys.path.insert(0, '/opt/trn_rl_repo')

from collections import deque
from contextlib import ExitStack

import numpy as np
import ml_dtypes

import concourse.bass as bass
import concourse.tile as tile
from concourse import bacc, mybir
from concourse.bass_utils import run_bass_kernel_spmd

P = 128
T = 2048          # tokens per core (one batch)
E = 1024          # embed
HPC = 8           # heads per core
D = 64            # head dim
GD = HPC * D      # 512 group dims per core
NE = E // P       # 8 contraction chunks
SLOT = D + 1      # v slot width: [v(64) | ones]
F32 = mybir.dt.float32
FPR = mybir.dt.float32r
BF = mybir.dt.bfloat16
F8 = mybir.dt.float8e4
EXP = mybir.ActivationFunctionType.Exp
MUL = mybir.AluOpType.mult
ADD = mybir.AluOpType.add
DR = mybir.MatmulPerfMode.DoubleRow

_CACHE = {}


def _build():
    nc = bacc.Bacc("TRN2", target_bir_lowering=False, debug=False)

    xtr = nc.dram_tensor("xt", [P, NE, T], FPR, kind="ExternalInput").ap()
    wqr = nc.dram_tensor("wq", [P, NE, GD], FPR, kind="ExternalInput").ap()
    wkr = nc.dram_tensor("wk", [P, NE, GD], FPR, kind="ExternalInput").ap()
    wvr = nc.dram_tensor("wv", [P, NE, GD], FPR, kind="ExternalInput").ap()
    wpr = nc.dram_tensor("wp", [P, 4, E], BF, kind="ExternalInput").ap()
    bqk = nc.dram_tensor("bqk", [2, P, 4], F32, kind="ExternalInput").ap()
    bvb = nc.dram_tensor("bvb", [P, GD], F32, kind="ExternalInput").ap()
    masks = nc.dram_tensor("masks", [P, P], F32, kind="ExternalInput").ap()
    ones = nc.dram_tensor("ones", [P, HPC], F32, kind="ExternalInput").ap()
    out = nc.dram_tensor("out", [T, E], F32, kind="ExternalOutput").ap()

    outr = out.rearrange("(to tp) o -> to tp o", tp=P)    # [16, 128, 1024]

    with tile.TileContext(nc) as tc, ExitStack() as ctx:
        w_pool = ctx.enter_context(tc.tile_pool(name="wts", bufs=1))
        qk_pool = ctx.enter_context(tc.tile_pool(name="qk", bufs=1))
        vaug_pool = ctx.enter_context(tc.tile_pool(name="vaug", bufs=1))
        const_pool = ctx.enter_context(tc.tile_pool(name="const", bufs=1))
        xs_pool = ctx.enter_context(tc.tile_pool(name="xs", bufs=4))
        q8t_pool = ctx.enter_context(tc.tile_pool(name="q8t", bufs=3))
        exp_pool = ctx.enter_context(tc.tile_pool(name="exps", bufs=3))
        ho_pool = ctx.enter_context(tc.tile_pool(name="ho", bufs=2))
        norm_pool = ctx.enter_context(tc.tile_pool(name="norm", bufs=2))
        nsc_pool = ctx.enter_context(tc.tile_pool(name="nsc", bufs=1))
        ot_pool = ctx.enter_context(tc.tile_pool(name="outs", bufs=2))
        psS = ctx.enter_context(tc.tile_pool(name="psS", bufs=2, space="PSUM"))
        psAV = ctx.enter_context(tc.tile_pool(name="psAV", bufs=1, space="PSUM"))
        psA = ctx.enter_context(tc.tile_pool(name="psA", bufs=2, space="PSUM"))

        wq_t = w_pool.tile([P, NE, GD], FPR, tag="wq", name="wq_t")
        wk_t = w_pool.tile([P, NE, GD], FPR, tag="wk", name="wk_t")
        wv_t = w_pool.tile([P, NE, GD], FPR, tag="wv", name="wv_t")
        wp_t = w_pool.tile([P, 4, E], BF, tag="wp", name="wp_t")
        # fp8 q/k in DoubleRow layout: partition 32h+p, ktile j <- dim 32j+p
        q8 = [qk_pool.tile([64, 2, T], F8, tag=f"q8{i}", name=f"q8{i}") for i in range(4)]
        k8 = [qk_pool.tile([64, 2, T], F8, tag=f"k8{i}", name=f"k8{i}") for i in range(4)]
        vaug = [vaug_pool.tile([P, HPC * SLOT], FPR, tag=f"va{t}", name=f"va{t}")
                for t in range(T // P)]

        bqk_t = const_pool.tile([P, 2, 4], F32, tag="bqk", name="bqk_t")
        nc.sync.dma_start(bqk_t[:], bqk.rearrange("b p m -> p b m"))
        bvb_t = const_pool.tile([P, GD], F32, tag="bvb", name="bvb_t")
        nc.sync.dma_start(bvb_t[:], bvb)
        masks_t = const_pool.tile([P, P], F32, tag="masks", name="masks_t")
        nc.sync.dma_start(masks_t[:], masks)
        onec_t = const_pool.tile([P, HPC], F32, tag="onec", name="onec_t")
        nc.sync.dma_start(onec_t[:], ones)

        def load_weights_head():
            nc.sync.dma_start(wq_t[:, :, bass.ts(0, P)], wqr[:, :, bass.ts(0, P)])
            nc.sync.dma_start(wk_t[:, :, bass.ts(0, P)], wkr[:, :, bass.ts(0, P)])

        def load_weights_rest():
            for half in range(2):
                esl = bass.ds(4 * half, 4)
                nc.sync.dma_start(wv_t[:, esl], wvr[:, esl])
            for m in range(1, 4):
                msl = bass.ts(m, P)
                nc.sync.dma_start(wq_t[:, :, msl], wqr[:, :, msl])
                nc.sync.dma_start(wk_t[:, :, msl], wkr[:, :, msl])
            nc.sync.dma_start(wp_t[:], wpr)

        def make_A_units(tb):
            """24 PE units (~0.85us each) projecting q,k,v for token block tb.
            The x-slice DMAs are emitted eagerly at make time (prefetch)."""
            tbsl = bass.ts(tb, 512)
            st = {}
            st['xs'] = [xs_pool.tile([P, 4, 512], FPR, tag="xs",
                                     name=f"xs{tb}_{h}") for h in range(2)]
            for h in range(2):
                for q in range(2):
                    nc.sync.dma_start(st['xs'][h][:, 2 * q:2 * q + 2],
                                      xtr[:, 4 * h + 2 * q:4 * h + 2 * q + 2, tbsl])

            units = []
            for m in range(4):
                for w_t, row, dst in ((wq_t, 0, q8), (wk_t, 1, k8)):
                    key = f"p{row}_{m}"

                    def u1(m=m, w_t=w_t, key=key):
                        pp = psA.tile([P, 512], F32, tag="pa", name=f"{key}_{tb}")
                        st[key] = pp
                        for e in range(4):
                            nc.tensor.matmul(pp[:], w_t[:, e, bass.ts(m, P)],
                                             st['xs'][0][:, e], start=(e == 0),
                                             stop=False)

                    def u2(m=m, w_t=w_t, row=row, dst=dst, key=key):
                        pp = st[key]
                        for e in range(4):
                            nc.tensor.matmul(pp[:], w_t[:, 4 + e, bass.ts(m, P)],
                                             st['xs'][1][:, e], start=False,
                                             stop=(e == 3))
                        tmp8 = q8t_pool.tile([P, 512], F8, tag="t8",
                                             name=f"t8{row}{tb}_{m}")
                        if row == 0:
                            nc.vector.tensor_scalar(tmp8[:], pp[:],
                                                    bqk_t[:, 0, m:m + 1],
                                                    0.125, ADD, MUL)
                        else:
                            nc.vector.tensor_scalar_add(tmp8[:], pp[:],
                                                        bqk_t[:, 1, m:m + 1])
                        for h in range(2):
                            for jt in range(2):
                                nc.sync.dma_start(
                                    dst[m][32 * h:32 * h + 32, jt, tbsl],
                                    tmp8[64 * h + 32 * jt:64 * h + 32 * jt + 32, :])

                    units += [u1, u2]
            for jj in range(4):
                key = f"v_{jj}"

                def v1(jj=jj, key=key):
                    vp = psA.tile([P, GD], F32, tag="pa", name=f"vp{tb}_{jj}")
                    st[key] = vp
                    for e in range(4):
                        nc.tensor.matmul(vp[:], st['xs'][0][:, e, bass.ts(jj, P)],
                                         wv_t[:, e], start=(e == 0), stop=False)

                def v2(jj=jj, key=key):
                    vp = st[key]
                    for e in range(4):
                        nc.tensor.matmul(vp[:], st['xs'][1][:, e, bass.ts(jj, P)],
                                         wv_t[:, 4 + e], start=False, stop=(e == 3))
                    t = 4 * tb + jj
                    va3 = vaug[t][:].rearrange("p (h s) -> p h s", s=SLOT)
                    nc.vector.tensor_copy(va3[:, :, D:D + 1], onec_t[:].unsqueeze(2))
                    nc.vector.tensor_tensor(va3[:, :, 0:D],
                                            vp[:].rearrange("p (h d) -> p h d", d=D),
                                            bvb_t[:].rearrange("p (h d) -> p h d", d=D),
                                            ADD)

                units += [v1, v2]
            return units

        def make_D_units(qb, hos):
            """8 PE units: partial out = concat(heads) @ Wp_g.T for tokens of qb."""
            units = []
            for tq in range(4):
                for nh in range(2):
                    def u(tq=tq, nh=nh):
                        dp = psA.tile([P, 512], F32, tag="pa",
                                      name=f"dp{qb}_{tq}_{nh}")
                        for db in range(4):
                            nc.tensor.matmul(dp[:], hos[db][:, bass.ts(tq, P)],
                                             wp_t[:, db, bass.ts(nh, 512)],
                                             start=(db == 0), stop=(db == 3))
                        ot = ot_pool.tile([P, 512], F32, tag="ot",
                                          name=f"ot{qb}_{tq}_{nh}")
                        nc.vector.tensor_copy(ot[:], dp[:])
                        nc.sync.dma_start(outr[4 * qb + tq, :, bass.ts(nh, 512)],
                                          ot[:])
                    units.append(u)
            return units

        fillers = deque()

        def fill(n):
            for _ in range(min(n, len(fillers))):
                fillers.popleft()()

        def emit_C(qb, stride, pre_hp=None):
            """attention for queries of block qb. 1-block software pipeline:
            S/exp of block kb+1 are emitted BEFORE the AV of block kb so the
            in-order PE queue never reaches an AV whose exp hasn't had a full
            block-time to run; a filler unit is popped every `stride` blocks
            to absorb the remaining ScalarE-vs-PE imbalance."""
            nkb = 4 * (qb + 1)
            qsl0 = 512 * qb
            hos = []

            def emit_s(hp, kb):
                j = kb - 4 * qb
                q0 = 128 * j if j > 0 else 0
                sp = psS.tile([P, 1024], F32, tag="sp", name=f"sp{qb}_{hp}_{kb}")
                for h in range(2):
                    nc.tensor.matmul(
                        sp[:, 512 * h + q0:512 * h + 512],
                        k8[hp][32 * h:32 * h + 32, :, bass.ts(kb, P)],
                        q8[hp][32 * h:32 * h + 32, :, qsl0 + q0:qsl0 + 512],
                        start=True, stop=True, perf_mode=DR)
                et = exp_pool.tile([P, 1024], FPR, tag="et", name=f"et{qb}_{hp}_{kb}")
                if q0 == 0:
                    nc.scalar.activation(et[:], sp[:], EXP)
                else:
                    nc.scalar.activation(et[:, q0:512], sp[:, q0:512], EXP)
                    nc.scalar.activation(et[:, 512 + q0:1024],
                                         sp[:, 512 + q0:1024], EXP)
                if j >= 0:
                    dsl = bass.ds(q0, P)
                    nc.vector.tensor_tensor(
                        et[:].rearrange("p (two n) -> p two n", two=2)[:, :, dsl],
                        et[:].rearrange("p (two n) -> p two n", two=2)[:, :, dsl],
                        masks_t[:, None, :].to_broadcast([P, 2, P]), MUL)
                return et, q0

            for hp in range(4):
                for u in (pre_hp or {}).get(hp, []):
                    u()
                avA = psAV.tile([P, 512], F32, tag="avA", name=f"avA{qb}_{hp}")
                avB = psAV.tile([P, 512], F32, tag="avB", name=f"avB{qb}_{hp}")
                pend = {0: emit_s(hp, 0)}
                for kb in range(nkb):
                    if kb + 1 < nkb:
                        pend[kb + 1] = emit_s(hp, kb + 1)
                    if kb % stride == stride - 1:
                        fill(1)
                    et, q0 = pend.pop(kb)
                    st = (kb == 0)
                    sp_ = (kb == nkb - 1)
                    vsl = vaug[kb][:].rearrange("p (hh s) -> p hh s", s=SLOT)
                    nc.tensor.matmul(avA[0:SLOT, q0:512], vsl[:, 2 * hp + 0],
                                     et[:, q0:512], start=st, stop=sp_)
                    nc.tensor.matmul(avB[0:SLOT, q0:512], vsl[:, 2 * hp + 1],
                                     et[:, 512 + q0:1024], start=st, stop=sp_)

                # normalize via ones-column denominator at partition 64
                ho = ho_pool.tile([P, 512], BF, tag=f"ho{hp}", name=f"ho{qb}_{hp}")
                avSA = norm_pool.tile([SLOT, 512], F32, tag="avSA", name=f"avSA{qb}_{hp}")
                nc.vector.tensor_copy(avSA[:], avA[0:SLOT, :])
                avSB = norm_pool.tile([SLOT, 512], F32, tag="avSB", name=f"avSB{qb}_{hp}")
                nc.vector.tensor_copy(avSB[:], avB[0:SLOT, :])
                d32 = nsc_pool.tile([32, 32], F32, tag="d32", name=f"d32{qb}_{hp}")
                nc.sync.dma_start(d32[:, 0:16], avSA[D:SLOT, :])
                nc.sync.dma_start(d32[:, 16:32], avSB[D:SLOT, :])
                nc.vector.reciprocal(d32[:], d32[:])
                rc0A = nsc_pool.tile([1, 512], F32, tag="rc0A", name=f"rA{qb}_{hp}")
                nc.sync.dma_start(rc0A[0:1, :], d32[:, 0:16])
                rc0B = nsc_pool.tile([1, 512], F32, tag="rc0B", name=f"rB{qb}_{hp}")
                nc.sync.dma_start(rc0B[0:1, :], d32[:, 16:32])
                bcA = nsc_pool.tile([D, 512], F32, tag="bcA", name=f"bA{qb}_{hp}")
                nc.gpsimd.partition_broadcast(bcA[:], rc0A[0:1, :], channels=D)
                bcB = nsc_pool.tile([D, 512], F32, tag="bcB", name=f"bB{qb}_{hp}")
                nc.gpsimd.partition_broadcast(bcB[:], rc0B[0:1, :], channels=D)
                nc.vector.tensor_tensor(ho[0:D, :], avSA[0:D, :], bcA[:], MUL)
                tmpB = nsc_pool.tile([D, 512], BF, tag="tmpB", name=f"tB{qb}_{hp}")
                nc.vector.tensor_tensor(tmpB[:], avSB[0:D, :], bcB[:], MUL)
                nc.sync.dma_start(ho[D:P, :], tmpB[:])
                hos.append(ho)
                fill(1)
            return hos

        units0 = make_A_units(0)   # xs(0) DMA queued before the weight bulk
        load_weights_head()
        # A0 unit layout: [qm0(2) km0(2) qm1(2) km1(2) ... v0(2) v1(2) v2(2) v3(2)]
        for u in units0[0:4]:
            u()                        # q/k for hp0 (shuffle DMAs beat the bulk)
        load_weights_rest()
        for u in units0[16:24]:
            u()                        # all of v
        fillers.extend(make_A_units(1))
        hos0 = emit_C(0, 2, pre_hp={1: units0[4:8], 2: units0[8:12],
                                    3: units0[12:16]})
        fill(99)                           # finish A(1) before C(1)
        fillers.extend(make_D_units(0, hos0))
        fillers.extend(make_A_units(2))
        hos1 = emit_C(1, 2)
        fill(99)
        fillers.extend(make_D_units(1, hos1))
        fillers.extend(make_A_units(3))
        hos2 = emit_C(2, 2)
        fill(99)
        fillers.extend(make_D_units(2, hos2))
        hos3 = emit_C(3, 4)
        fill(99)
        for u in make_D_units(3, hos3):
            u()

    nc.compile()
    return nc


def _in_maps(x, Wq, bq, Wk, bk, Wv, bv, Wp, bp):
    maskv = (np.arange(P)[:, None] <= np.arange(P)[None, :]).astype(np.float32)
    maps = []
    for c in range(8):
        b, g = divmod(c, 2)
        gs = slice(512 * g, 512 * (g + 1))
        maps.append({
            "xt": np.ascontiguousarray(
                x[b].T.reshape(NE, P, T).transpose(1, 0, 2)),
            "wq": np.ascontiguousarray(
                Wq[gs, :].T.reshape(NE, P, GD).transpose(1, 0, 2)),
            "wk": np.ascontiguousarray(
                Wk[gs, :].T.reshape(NE, P, GD).transpose(1, 0, 2)),
            "wv": np.ascontiguousarray(
                Wv[gs, :].T.reshape(NE, P, GD).transpose(1, 0, 2)),
            "wp": np.ascontiguousarray(
                Wp[:, gs].T.reshape(4, P, E).transpose(1, 0, 2)).astype(
                    ml_dtypes.bfloat16),
            "bqk": np.stack([bq[gs].reshape(4, P).T, bk[gs].reshape(4, P).T]),
            "bvb": np.broadcast_to(bv[gs], (P, 512)).astype(np.float32).copy(),
            "masks": maskv,
            "ones": np.ones((P, HPC), np.float32),
        })
    return maps


def kernel(x, Wq, bq, Wk, bk, Wv, bv, Wp, bp, _trace=False):
    if "nc" not in _CACHE:
        _CACHE["nc"] = _build()
    nc = _CACHE["nc"]
    res = run_bass_kernel_spmd(nc, _in_maps(x, Wq, bq, Wk, bk, Wv, bv, Wp, bp),
                               list(range(8)), trace=_trace)
    _CACHE["last_result"] = res
    out = np.empty((4, T, E), np.float32)
    for b in range(4):
        out[b] = res.results[2 * b]["out"] + res.results[2 * b + 1]["out"] + bp
    return out



# revision 2
# speedup vs baseline: 1.2456x; 1.2456x over previous
"""Causal self-attention (B=4, T=2048, E=1024, H=16, D=64) on 8 TRN2 NeuronCores.

Sharding: core c -> batch b=c//2, head-group g=c%2 (8 heads each).

v5 = v4 with the PE fed at ~1 col/cycle everywhere and cheaper projections:
- Q/K projections in fp8e4 DoubleRow with contraction 256 ([128,2,...]
  stationary): half the column-visits of the old fp32r path. x and Wq/Wk
  ship as fp8 (W prescaled x16, rescaled in the quantize step); q/k then
  quantize to the same fp8 DoubleRow S layout as v4.
- V projection, attention AV, and the output projection run in bf16
  (fp8 failed the accuracy budget for v/out; bf16 matches fp32r's
  1 col/cycle rate but halves DMA/SBUF and LDWEIGHTS cost).
- exp output et is bf16 (halves SBUF + 2x faster mask TT); scalar exp
  rate itself is input-bound so unchanged.
- x ships as fp8 (2MB) + bf16 (4MB) instead of fp32 (8MB): faster start.
- A/D filler weave and 1-block S/exp lookahead kept from v4; a few D
  fillers are reserved for the post-attention normalize tail.
Host: out[b] = partial[2b] + partial[2b+1] + bp.
"""
import sys

if '/opt/trn_rl_repo' not in sys.path:
    sys.path.insert(0, '/opt/trn_rl_repo')

from collections import deque
from contextlib import ExitStack

import numpy as np
import ml_dtypes

import concourse.bass as bass
import concourse.tile as tile
from concourse import bacc, mybir
from concourse.bass_utils import run_bass_kernel_spmd

P = 128
T = 2048          # tokens per core (one batch)
E = 1024          # embed
HPC = 8           # heads per core
D = 64            # head dim
GD = HPC * D      # 512 group dims per core
NE = E // P       # 8 contraction chunks (bf16 path)
NC = 4            # 256-deep contraction chunks (fp8 DR path)
SLOT = D + 1      # v slot width: [v(64) | ones]
F32 = mybir.dt.float32
BF = mybir.dt.bfloat16
F8 = mybir.dt.float8e4
EXP = mybir.ActivationFunctionType.Exp
MUL = mybir.AluOpType.mult
ADD = mybir.AluOpType.add
DR = mybir.MatmulPerfMode.DoubleRow
WS = 16.0         # host prescale on Wq/Wk before fp8 cast

_CACHE = {}


def _build():
    nc = bacc.Bacc("TRN2", target_bir_lowering=False, debug=False)

    x8r = nc.dram_tensor("x8", [P, 4, NC, 2, 512], F8, kind="ExternalInput").ap()
    xbr = nc.dram_tensor("xb", [P, 4, NE, 512], BF, kind="ExternalInput").ap()
    wq8 = nc.dram_tensor("wq8", [P, NC, 2, GD], F8, kind="ExternalInput").ap()
    wk8 = nc.dram_tensor("wk8", [P, NC, 2, GD], F8, kind="ExternalInput").ap()
    wvr = nc.dram_tensor("wv", [P, NE, GD], BF, kind="ExternalInput").ap()
    wpr = nc.dram_tensor("wp", [P, 4, E], BF, kind="ExternalInput").ap()
    bqk = nc.dram_tensor("bqk", [2, P, 4], F32, kind="ExternalInput").ap()
    bvb = nc.dram_tensor("bvb", [P, GD], F32, kind="ExternalInput").ap()
    masks = nc.dram_tensor("masks", [P, P], F32, kind="ExternalInput").ap()
    ones = nc.dram_tensor("ones", [P, HPC], F32, kind="ExternalInput").ap()
    out = nc.dram_tensor("out", [T, E], F32, kind="ExternalOutput").ap()

    outr = out.rearrange("(to tp) o -> to tp o", tp=P)    # [16, 128, 1024]

    with tile.TileContext(nc) as tc, ExitStack() as ctx:
        w_pool = ctx.enter_context(tc.tile_pool(name="wts", bufs=1))
        qk_pool = ctx.enter_context(tc.tile_pool(name="qk", bufs=1))
        vaug_pool = ctx.enter_context(tc.tile_pool(name="vaug", bufs=1))
        const_pool = ctx.enter_context(tc.tile_pool(name="const", bufs=1))
        x8_pool = ctx.enter_context(tc.tile_pool(name="x8s", bufs=2))
        xs_pool = ctx.enter_context(tc.tile_pool(name="xs", bufs=4))
        q8t_pool = ctx.enter_context(tc.tile_pool(name="q8t", bufs=3))
        exp_pool = ctx.enter_context(tc.tile_pool(name="exps", bufs=3))
        ho_pool = ctx.enter_context(tc.tile_pool(name="ho", bufs=2))
        norm_pool = ctx.enter_context(tc.tile_pool(name="norm", bufs=2))
        nsc_pool = ctx.enter_context(tc.tile_pool(name="nsc", bufs=1))
        ot_pool = ctx.enter_context(tc.tile_pool(name="outs", bufs=2))
        psS = ctx.enter_context(tc.tile_pool(name="psS", bufs=2, space="PSUM"))
        psAV = ctx.enter_context(tc.tile_pool(name="psAV", bufs=1, space="PSUM"))
        psA = ctx.enter_context(tc.tile_pool(name="psA", bufs=2, space="PSUM"))

        wq_t = w_pool.tile([P, NC, 2, GD], F8, tag="wq", name="wq_t")
        wk_t = w_pool.tile([P, NC, 2, GD], F8, tag="wk", name="wk_t")
        wv_t = w_pool.tile([P, NE, GD], BF, tag="wv", name="wv_t")
        wp_t = w_pool.tile([P, 4, E], BF, tag="wp", name="wp_t")
        # fp8 q/k in DoubleRow layout: partition 32h+p, ktile j <- dim 32j+p
        q8 = [qk_pool.tile([64, 2, T], F8, tag=f"q8{i}", name=f"q8{i}") for i in range(4)]
        k8 = [qk_pool.tile([64, 2, T], F8, tag=f"k8{i}", name=f"k8{i}") for i in range(4)]
        vaug = [vaug_pool.tile([P, HPC * SLOT], BF, tag=f"va{t}", name=f"va{t}")
                for t in range(T // P)]

        bqk_t = const_pool.tile([P, 2, 4], F32, tag="bqk", name="bqk_t")
        nc.sync.dma_start(bqk_t[:], bqk.rearrange("b p m -> p b m"))
        bvb_t = const_pool.tile([P, GD], F32, tag="bvb", name="bvb_t")
        nc.sync.dma_start(bvb_t[:], bvb)
        masks_t = const_pool.tile([P, P], F32, tag="masks", name="masks_t")
        nc.sync.dma_start(masks_t[:], masks)
        onec_t = const_pool.tile([P, HPC], F32, tag="onec", name="onec_t")
        nc.sync.dma_start(onec_t[:], ones)

        def load_weights_head():
            nc.sync.dma_start(wq_t[:], wq8)
            nc.sync.dma_start(wk_t[:], wk8)

        def load_weights_rest():
            for half in range(2):
                esl = bass.ds(4 * half, 4)
                nc.sync.dma_start(wv_t[:, esl], wvr[:, esl])
            nc.sync.dma_start(wp_t[:], wpr)

        def make_A_units(tb):
            """16 PE units (~0.85us each) projecting q,k,v for token block tb.
            The x-slice DMAs are emitted eagerly at make time (prefetch)."""
            st = {}
            st['x8'] = x8_pool.tile([P, NC, 2, 512], F8, tag="x8",
                                    name=f"x8_{tb}")
            nc.sync.dma_start(st['x8'][:], x8r[:, tb])
            st['xs'] = [xs_pool.tile([P, 4, 512], BF, tag="xs",
                                     name=f"xs{tb}_{h}") for h in range(2)]
            for h in range(2):
                nc.sync.dma_start(st['xs'][h][:], xbr[:, tb, 4 * h:4 * h + 4])

            units = []
            for m in range(4):
                for w_t, row, dst in ((wq_t, 0, q8), (wk_t, 1, k8)):
                    def u(m=m, w_t=w_t, row=row, dst=dst):
                        pp = psA.tile([P, 512], F32, tag="pa",
                                      name=f"p{row}_{m}_{tb}")
                        msl = bass.ts(m, P)
                        for c in range(NC):
                            nc.tensor.matmul(pp[:], w_t[:, c, :, msl],
                                             st['x8'][:, c], start=(c == 0),
                                             stop=(c == NC - 1), perf_mode=DR)
                        tmp8 = q8t_pool.tile([P, 512], F8, tag="t8",
                                             name=f"t8{row}{tb}_{m}")
                        if row == 0:
                            nc.vector.tensor_scalar(tmp8[:], pp[:],
                                                    bqk_t[:, 0, m:m + 1],
                                                    0.125 / WS, ADD, MUL)
                        else:
                            nc.vector.tensor_scalar(tmp8[:], pp[:],
                                                    bqk_t[:, 1, m:m + 1],
                                                    1.0 / WS, ADD, MUL)
                        tbsl = bass.ts(tb, 512)
                        for h in range(2):
                            for jt in range(2):
                                nc.sync.dma_start(
                                    dst[m][32 * h:32 * h + 32, jt, tbsl],
                                    tmp8[64 * h + 32 * jt:64 * h + 32 * jt + 32, :])
                    units.append(u)
            for jj in range(4):
                key = f"v_{jj}"

                def v1(jj=jj, key=key):
                    vp = psA.tile([P, GD], F32, tag="pa", name=f"vp{tb}_{jj}")
                    st[key] = vp
                    for e in range(4):
                        nc.tensor.matmul(vp[:], st['xs'][0][:, e, bass.ts(jj, P)],
                                         wv_t[:, e], start=(e == 0), stop=False)

                def v2(jj=jj, key=key):
                    vp = st[key]
                    for e in range(4):
                        nc.tensor.matmul(vp[:], st['xs'][1][:, e, bass.ts(jj, P)],
                                         wv_t[:, 4 + e], start=False, stop=(e == 3))
                    t = 4 * tb + jj
                    va3 = vaug[t][:].rearrange("p (h s) -> p h s", s=SLOT)
                    nc.vector.tensor_copy(va3[:, :, D:D + 1], onec_t[:].unsqueeze(2))
                    nc.vector.tensor_tensor(va3[:, :, 0:D],
                                            vp[:].rearrange("p (h d) -> p h d", d=D),
                                            bvb_t[:].rearrange("p (h d) -> p h d", d=D),
                                            ADD)

                units += [v1, v2]
            return units

        def make_D_units(qb, hos):
            """8 PE units: partial out = concat(heads) @ Wp_g.T for tokens of qb."""
            units = []
            for tq in range(4):
                for nh in range(2):
                    def u(tq=tq, nh=nh):
                        dp = psA.tile([P, 512], F32, tag="pa",
                                      name=f"dp{qb}_{tq}_{nh}")
                        for db in range(4):
                            nc.tensor.matmul(dp[:], hos[db][:, bass.ts(tq, P)],
                                             wp_t[:, db, bass.ts(nh, 512)],
                                             start=(db == 0), stop=(db == 3))
                        ot = ot_pool.tile([P, 512], F32, tag="ot",
                                          name=f"ot{qb}_{tq}_{nh}")
                        nc.vector.tensor_copy(ot[:], dp[:])
                        nc.sync.dma_start(outr[4 * qb + tq, :, bass.ts(nh, 512)],
                                          ot[:])
                    units.append(u)
            return units

        fillers = deque()

        def fill(n):
            for _ in range(min(n, len(fillers))):
                fillers.popleft()()

        def emit_C(qb, stride, pre_hp=None):
            """attention for queries of block qb. 1-block software pipeline:
            S/exp of block kb+1 are emitted BEFORE the AV of block kb so the
            in-order PE queue never reaches an AV whose exp hasn't had a full
            block-time to run; a filler unit is popped every `stride` blocks
            to absorb the remaining ScalarE-vs-PE imbalance."""
            nkb = 4 * (qb + 1)
            qsl0 = 512 * qb
            hos = []

            def emit_s(hp, kb):
                j = kb - 4 * qb
                q0 = 128 * j if j > 0 else 0
                sp = psS.tile([P, 1024], F32, tag="sp", name=f"sp{qb}_{hp}_{kb}")
                for h in range(2):
                    nc.tensor.matmul(
                        sp[:, 512 * h + q0:512 * h + 512],
                        k8[hp][32 * h:32 * h + 32, :, bass.ts(kb, P)],
                        q8[hp][32 * h:32 * h + 32, :, qsl0 + q0:qsl0 + 512],
                        start=True, stop=True, perf_mode=DR)
                et = exp_pool.tile([P, 1024], BF, tag="et", name=f"et{qb}_{hp}_{kb}")
                if q0 == 0:
                    nc.scalar.activation(et[:], sp[:], EXP)
                else:
                    nc.scalar.activation(et[:, q0:512], sp[:, q0:512], EXP)
                    nc.scalar.activation(et[:, 512 + q0:1024],
                                         sp[:, 512 + q0:1024], EXP)
                if j >= 0:
                    dsl = bass.ds(q0, P)
                    nc.vector.tensor_tensor(
                        et[:].rearrange("p (two n) -> p two n", two=2)[:, :, dsl],
                        et[:].rearrange("p (two n) -> p two n", two=2)[:, :, dsl],
                        masks_t[:, None, :].to_broadcast([P, 2, P]), MUL)
                return et, q0

            for hp in range(4):
                for u in (pre_hp or {}).get(hp, []):
                    u()
                avA = psAV.tile([P, 512], F32, tag="avA", name=f"avA{qb}_{hp}")
                avB = psAV.tile([P, 512], F32, tag="avB", name=f"avB{qb}_{hp}")
                pend = {0: emit_s(hp, 0)}
                for kb in range(nkb):
                    if kb + 1 < nkb:
                        pend[kb + 1] = emit_s(hp, kb + 1)
                    if kb % stride == stride - 1:
                        fill(1)
                    et, q0 = pend.pop(kb)
                    st = (kb == 0)
                    sp_ = (kb == nkb - 1)
                    vsl = vaug[kb][:].rearrange("p (hh s) -> p hh s", s=SLOT)
                    nc.tensor.matmul(avA[0:SLOT, q0:512], vsl[:, 2 * hp + 0],
                                     et[:, q0:512], start=st, stop=sp_)
                    nc.tensor.matmul(avB[0:SLOT, q0:512], vsl[:, 2 * hp + 1],
                                     et[:, 512 + q0:1024], start=st, stop=sp_)

                # normalize via ones-column denominator at partition 64
                ho = ho_pool.tile([P, 512], BF, tag=f"ho{hp}", name=f"ho{qb}_{hp}")
                avSA = norm_pool.tile([SLOT, 512], F32, tag="avSA", name=f"avSA{qb}_{hp}")
                nc.vector.tensor_copy(avSA[:], avA[0:SLOT, :])
                avSB = norm_pool.tile([SLOT, 512], F32, tag="avSB", name=f"avSB{qb}_{hp}")
                nc.vector.tensor_copy(avSB[:], avB[0:SLOT, :])
                d32 = nsc_pool.tile([32, 32], F32, tag="d32", name=f"d32{qb}_{hp}")
                nc.sync.dma_start(d32[:, 0:16], avSA[D:SLOT, :])
                nc.sync.dma_start(d32[:, 16:32], avSB[D:SLOT, :])
                nc.vector.reciprocal(d32[:], d32[:])
                rc0A = nsc_pool.tile([1, 512], F32, tag="rc0A", name=f"rA{qb}_{hp}")
                nc.sync.dma_start(rc0A[0:1, :], d32[:, 0:16])
                rc0B = nsc_pool.tile([1, 512], F32, tag="rc0B", name=f"rB{qb}_{hp}")
                nc.sync.dma_start(rc0B[0:1, :], d32[:, 16:32])
                bcA = nsc_pool.tile([D, 512], F32, tag="bcA", name=f"bA{qb}_{hp}")
                nc.gpsimd.partition_broadcast(bcA[:], rc0A[0:1, :], channels=D)
                bcB = nsc_pool.tile([D, 512], F32, tag="bcB", name=f"bB{qb}_{hp}")
                nc.gpsimd.partition_broadcast(bcB[:], rc0B[0:1, :], channels=D)
                nc.vector.tensor_tensor(ho[0:D, :], avSA[0:D, :], bcA[:], MUL)
                tmpB = nsc_pool.tile([D, 512], BF, tag="tmpB", name=f"tB{qb}_{hp}")
                nc.vector.tensor_tensor(tmpB[:], avSB[0:D, :], bcB[:], MUL)
                nc.sync.dma_start(ho[D:P, :], tmpB[:])
                hos.append(ho)
                fill(1)
            return hos

        units0 = make_A_units(0)   # xs(0) DMA queued before the weight bulk
        load_weights_head()
        # A0 unit layout: [q_m0 k_m0 q_m1 k_m1 ... v0(2) v1(2) v2(2) v3(2)]
        for u in units0[0:2]:
            u()                        # q/k for hp0 (shuffle DMAs beat the bulk)
        load_weights_rest()
        for u in units0[8:16]:
            u()                        # all of v
        fillers.extend(make_A_units(1))
        hos0 = emit_C(0, 2, pre_hp={1: units0[2:4], 2: units0[4:6],
                                    3: units0[6:8]})
        fill(99)                           # finish A(1) before C(1)
        fillers.extend(make_D_units(0, hos0))
        fillers.extend(make_A_units(2))
        hos1 = emit_C(1, 2)
        fill(99)
        fillers.extend(make_D_units(1, hos1))
        fillers.extend(make_A_units(3))
        hos2 = emit_C(2, 2)
        fill(99)
        fillers.extend(make_D_units(2, hos2))
        hos3 = emit_C(3, 6)                # leave ~4 D(2) fillers for the tail
        fill(99)
        for u in make_D_units(3, hos3):
            u()

    nc.compile()
    return nc


def _in_maps(x, Wq, bq, Wk, bk, Wv, bv, Wp, bp):
    maskv = (np.arange(P)[:, None] <= np.arange(P)[None, :]).astype(np.float32)
    f8 = ml_dtypes.float8_e4m3
    bf = ml_dtypes.bfloat16
    maps = []
    for c in range(8):
        b, g = divmod(c, 2)
        gs = slice(512 * g, 512 * (g + 1))
        xT = x[b].T                                   # [E, T]
        x8 = np.ascontiguousarray(
            xT.reshape(NC, 2, P, 4, 512).transpose(2, 3, 0, 1, 4)).astype(f8)
        xb = np.ascontiguousarray(
            xT.reshape(NE, P, 4, 512).transpose(1, 2, 0, 3)).astype(bf)
        wq8 = np.ascontiguousarray(
            (Wq[gs, :].T * WS).reshape(NC, 2, P, GD).transpose(2, 0, 1, 3)
        ).astype(f8)
        wk8 = np.ascontiguousarray(
            (Wk[gs, :].T * WS).reshape(NC, 2, P, GD).transpose(2, 0, 1, 3)
        ).astype(f8)
        maps.append({
            "x8": x8,
            "xb": xb,
            "wq8": wq8,
            "wk8": wk8,
            "wv": np.ascontiguousarray(
                Wv[gs, :].T.reshape(NE, P, GD).transpose(1, 0, 2)).astype(bf),
            "wp": np.ascontiguousarray(
                Wp[:, gs].T.reshape(4, P, E).transpose(1, 0, 2)).astype(bf),
            "bqk": np.stack([WS * bq[gs].reshape(4, P).T,
                             WS * bk[gs].reshape(4, P).T]).astype(np.float32),
            "bvb": np.broadcast_to(bv[gs], (P, 512)).astype(np.float32).copy(),
            "masks": maskv,
            "ones": np.ones((P, HPC), np.float32),
        })
    return maps


def kernel(x, Wq, bq, Wk, bk, Wv, bv, Wp, bp, _trace=False):
    if "nc" not in _CACHE:
        _CACHE["nc"] = _build()
    nc = _CACHE["nc"]
    res = run_bass_kernel_spmd(nc, _in_maps(x, Wq, bq, Wk, bk, Wv, bv, Wp, bp),
                               list(range(8)), trace=_trace)
    _CACHE["last_result"] = res
    out = np.empty((4, T, E), np.float32)
    for b in range(4):
        out[b] = res.results[2 * b]["out"] + res.results[2 * b + 1]["out"] + bp
    return out
